# revision 1
# baseline (speedup 1.0000x reference)
"""Trainium2 Bass kernel for AttentionWithRoPE, head-sharded across 8 NeuronCores.

Reference computation (fp32):
    q = (x @ Wq) * Dh^-0.5, rope'd; k = (x @ Wk), rope'd; v = x @ Wv
    out = softmax(q k^T * Dh^-0.5) v ;  final = out @ Wo + bo

Sharding: tensor-parallel over heads. Each core owns 2 of 16 heads: it gets
the full x (pre-transposed to [D, B*N] on host), its column slices of
Wq/Wk/Wv, its row slice of Wo, and returns a partial [B*N, D] output that the
host sums over cores (+ bo).

Device layout choices:
  - Q^T/K^T are produced directly in [Dh, n] layout (D-contraction with x^T as
    the moving operand), so attention scores can be computed transposed
    (S^T[m, n], K stationary / Q moving) and the exp'd probabilities feed the
    attn@V matmul as the moving operand with V (natural [m, Dh] layout) as
    stationary -- no on-chip transposes anywhere.
  - RoPE pairs (even, odd) are separated by permuting the columns of Wq/Wk
    per head on the host so that the (real, imag) partners of each rotation
    pair sit exactly 16 partitions apart within the same 32-partition
    quadrant. The partner swap is then a legal DVE stream_shuffle (rotation
    by 16 inside each quadrant); RoPE becomes shuffle + 3 vector ops per
    tile with host-prepared factor tensors A (rr replicated) and B (+-ri).
    Scores are permutation-invariant since q and k share the permutation;
    the total 1/Dh score scale is folded into the q-rope factors.
  - Softmax denominators: P^T column sums = (chain of DVE adds over the 16
    m-chunk tiles) then a ones-vector matmul on the PE for the partition-dim
    reduction; reciprocal on DVE; broadcast back over partitions with a K=1
    ones-row matmul on the PE.
  - All matmul operands fp16 (PE runs fp16 at full rate; fp32 would be 4x
    slower), accumulation in fp32 PSUM. Partial outputs returned fp16.
"""

import os
import sys

for _p in ("/opt/trn_rl_repo", "/root/.axon_site/_ro/trn_rl_repo"):
    if os.path.isdir(_p) and _p not in sys.path:
        sys.path.insert(0, _p)

import numpy as np
from contextlib import ExitStack

import concourse.bass as bass
import concourse.bacc as bacc
import concourse.tile as tile
from concourse import mybir
from concourse.bass_utils import run_bass_kernel_spmd

F16 = mybir.dt.float16
F32 = mybir.dt.float32
AF = mybir.ActivationFunctionType

N_CORES = 8
B, N, D, H, Dh = 2, 2048, 2048, 16, 128
HL = H // N_CORES          # heads per core
DHL = HL * Dh              # 256 local head dims
BN = B * N                 # 4096
DCH = D // 128             # 16 contraction chunks
NBLK = BN // 512           # 8 projection column blocks
MCH = N // 128             # 16 key chunks per sequence
NCK = N // 512             # 4 query chunks per sequence

_CACHE = {}
_PHASE_MARKS = {}


def _build_nc(loop_n=1):
    nc = bacc.Bacc(trn_type="TRN2", target_bir_lowering=False, debug=False)

    xt_d = nc.dram_tensor("xt", [D, BN], F16, kind="ExternalInput")
    wq_d = nc.dram_tensor("wq", [D, DHL], F16, kind="ExternalInput")
    wk_d = nc.dram_tensor("wk", [D, DHL], F16, kind="ExternalInput")
    wv_d = nc.dram_tensor("wv", [D, DHL], F16, kind="ExternalInput")
    wo_d = nc.dram_tensor("wo", [DHL, D], F16, kind="ExternalInput")
    rope_d = nc.dram_tensor("rope", [2 * B * 2, 128, N], F16, kind="ExternalInput")
    out_d = nc.dram_tensor("out", [BN, D], F16, kind="ExternalOutput")

    xt_v = xt_d.ap().rearrange("(c p) n -> p c n", p=128)       # [128, 16, 4096]
    w_views = {
        "wq": wq_d.ap().rearrange("(c p) m -> p c m", p=128),   # [128, 16, 256]
        "wk": wk_d.ap().rearrange("(c p) m -> p c m", p=128),
        "wv": wv_d.ap().rearrange("(c p) m -> p c m", p=128),
    }
    wo_v = wo_d.ap().rearrange("(j p) d -> p j d", p=128)       # [128, 2, 2048]
    rope_v = rope_d.ap()                                        # [8, 128, 2048]
    out_v = out_d.ap().rearrange("(cb p) d -> cb p d", p=128)   # [32, 128, 2048]

    with tile.TileContext(nc) as tc:
        with ExitStack() as ctx:
            consts = ctx.enter_context(tc.tile_pool(name="consts", bufs=1))
            qtkt = ctx.enter_context(tc.tile_pool(name="qtkt", bufs=1))
            vres = ctx.enter_context(tc.tile_pool(name="vres", bufs=1))
            xin = ctx.enter_context(tc.tile_pool(name="xin", bufs=2))
            ropein = ctx.enter_context(tc.tile_pool(name="ropein", bufs=2))
            tmps = ctx.enter_context(tc.tile_pool(name="tmps", bufs=3))
            ptile = ctx.enter_context(tc.tile_pool(name="ptile", bufs=12))
            dacc = ctx.enter_context(tc.tile_pool(name="dacc", bufs=2))
            smalls = ctx.enter_context(tc.tile_pool(name="smalls", bufs=2))
            rbcp = ctx.enter_context(tc.tile_pool(name="rbcp", bufs=2))
            otbuf = ctx.enter_context(tc.tile_pool(name="otbuf", bufs=3))
            obuf = ctx.enter_context(tc.tile_pool(name="obuf", bufs=4))

            psa = ctx.enter_context(tc.tile_pool(name="psa", bufs=2, space="PSUM"))
            psb = ctx.enter_context(tc.tile_pool(name="psb", bufs=3, space="PSUM"))
            psc = ctx.enter_context(tc.tile_pool(name="psc", bufs=1, space="PSUM"))

            # ---- resident weights / constants ----
            w_sb = {}
            for wname in ("wq", "wk", "wv"):
                w_sb[wname] = consts.tile([128, DCH, DHL], F16, name=wname)

            def _load_w(wname):
                for dq in range(4):
                    nc.sync.dma_start(
                        w_sb[wname][:, dq * 4:(dq + 1) * 4, :],
                        w_views[wname][:, dq * 4:(dq + 1) * 4, :],
                    )
            _load_w("wq")
            wo_sb = consts.tile([128, HL, D], F16, name="wo")
            if loop_n > 1:
                nc.sync.dma_start(wo_sb[:], wo_v)
            ones_col = consts.tile([128, 1], F16, name="ones_col")
            nc.vector.memset(ones_col[:], 1.0)
            ones_row = consts.tile([1, 128], F16, name="ones_row")
            nc.vector.memset(ones_row[:], 1.0)
            swap_mask = [(i + 16) % 32 for i in range(32)]

            qt_sb = qtkt.tile([128, HL, BN], F16, name="qt")
            kt_sb = qtkt.tile([128, HL, BN], F16, name="kt")
            v_sb = vres.tile([128, BN // 128, DHL], F16, name="v")

            # ---- phase 1: projections + rope ----
            import contextlib
            loop_cm = tc.For_i(0, loop_n, 1) if loop_n > 1 else contextlib.nullcontext()
            with loop_cm:
              for blk in range(NBLK):
                  b = blk // (NBLK // B)
                  c0 = (blk % (NBLK // B)) * 512
                  xblk = xin.tile([128, DCH, 512], F16, name="xblk")
                  for dq in range(4):
                      nc.sync.dma_start(
                          xblk[:, dq * 4:(dq + 1) * 4, :],
                          xt_v[:, dq * 4:(dq + 1) * 4, blk * 512:(blk + 1) * 512],
                      )
                  rblk = ropein.tile([128, 4, 512], F16, name="rblk")
                  nc.sync.dma_start(
                      rblk[:], rope_v[4 * b:4 * b + 4, :, c0:c0 + 512].rearrange("r p n -> p r n")
                  )
                  if blk == 0:
                      _load_w("wk")
                      _load_w("wv")

                  for wname, dst_sb, ra, rb_ in (
                      ("wq", qt_sb, 0, 1),
                      ("wk", kt_sb, 2, 3),
                  ):
                      for j in range(HL):
                          ps = psa.tile([128, 512], F32, name="pp")
                          for dc in range(DCH):
                              nc.tensor.matmul(
                                  ps[:],
                                  w_sb[wname][:, dc, j * 128:(j + 1) * 128],
                                  xblk[:, dc, :],
                                  start=(dc == 0),
                                  stop=(dc == DCH - 1),
                              )
                          raw = tmps.tile([128, 512], F16, name="raw")
                          nc.scalar.copy(raw[:], ps[:])
                          t2 = tmps.tile([128, 512], F16, name="t2")
                          nc.vector.stream_shuffle(t2[:], raw[:], swap_mask)
                          nc.vector.tensor_mul(t2[:], t2[:], rblk[:, rb_, :])
                          nc.vector.tensor_mul(raw[:], raw[:], rblk[:, ra, :])
                          nc.vector.tensor_add(
                              dst_sb[:, j, blk * 512:(blk + 1) * 512], raw[:], t2[:]
                          )

                  for mc in range(4):
                      psv = psb.tile([128, DHL], F32, name="pb")
                      for dc in range(DCH):
                          nc.tensor.matmul(
                              psv[:],
                              xblk[:, dc, mc * 128:(mc + 1) * 128],
                              w_sb["wv"][:, dc, :],
                              start=(dc == 0),
                              stop=(dc == DCH - 1),
                          )
                      nc.scalar.copy(v_sb[:, blk * 4 + mc, :], psv[:])

              if loop_n == 1:
                  nc.sync.dma_start(wo_sb[:], wo_v)
              _PHASE_MARKS['end_phase1'] = int(nc.get_next_instruction_name()[2:])
              # ---- phase 2+3 per batch ----
              for b in range(B):
                  ot_tiles = [otbuf.tile([128, N], F16, name="ot") for _ in range(HL)]
                  for nck in range(NCK):
                      nq0 = b * N + nck * 512
                      for j in range(HL):
                          ot = ot_tiles[j]
                          pts = []
                          for mc2 in range(MCH // 2):
                              sp = psa.tile([128, 1024], F32, name="pp")
                              for half in range(2):
                                  mc = 2 * mc2 + half
                                  nc.tensor.matmul(
                                      sp[:, half * 512:(half + 1) * 512],
                                      kt_sb[:, j, b * N + mc * 128:b * N + (mc + 1) * 128],
                                      qt_sb[:, j, nq0:nq0 + 512],
                                      start=True,
                                      stop=True,
                                  )
                              pt = ptile.tile([128, 1024], F16, name="pt")
                              nc.scalar.activation(pt[:], sp[:], AF.Exp)
                              pts.append(pt)

                          op = psb.tile([128, 512], F32, name="pb")
                          for mc in range(MCH):
                              nc.tensor.matmul(
                                  op[:],
                                  v_sb[:, b * MCH + mc, j * 128:(j + 1) * 128],
                                  pts[mc // 2][:, (mc % 2) * 512:(mc % 2 + 1) * 512],
                                  start=(mc == 0),
                                  stop=(mc == MCH - 1),
                              )

                          acc = dacc.tile([128, 1024], F16, name="acc")
                          nc.vector.tensor_add(acc[:], pts[0][:], pts[1][:])
                          for mc2 in range(2, MCH // 2):
                              nc.vector.tensor_add(acc[:], acc[:], pts[mc2][:])
                          nc.vector.tensor_add(
                              acc[:, 0:512], acc[:, 0:512], acc[:, 512:1024]
                          )
                          dps = psc.tile([1, 512], F32, name="pc")
                          nc.tensor.matmul(
                              dps[:], ones_col[:], acc[:, 0:512], start=True, stop=True
                          )
                          rc32 = smalls.tile([1, 512], F32, name="rc32")
                          nc.vector.reciprocal(rc32[:], dps[:])
                          rc16 = smalls.tile([1, 512], F16, name="rc16")
                          nc.vector.tensor_copy(rc16[:], rc32[:])
                          bps = psc.tile([128, 512], F32, name="pc")
                          nc.tensor.matmul(bps[:], ones_row[:], rc16[:], start=True, stop=True)
                          rbc = rbcp.tile([128, 512], F16, name="rbc")
                          nc.vector.tensor_copy(rbc[:], bps[:])
                          nc.vector.tensor_mul(ot[:, nck * 512:(nck + 1) * 512], op[:], rbc[:])

                      # output projection for the n-range this nck covers
                      for nck2 in range(4):
                          ncol = nck * 4 + nck2
                          cb = b * (N // 128) + ncol
                          for dcol in range(D // 512):
                              ops3 = psb.tile([128, 512], F32, name="pb")
                              for j in range(HL):
                                  nc.tensor.matmul(
                                      ops3[:],
                                      ot_tiles[j][:, ncol * 128:(ncol + 1) * 128],
                                      wo_sb[:, j, dcol * 512:(dcol + 1) * 512],
                                      start=(j == 0),
                                      stop=(j == HL - 1),
                                  )
                              ob = obuf.tile([128, 512], F16, name="ob")
                              if dcol % 2 == 0:
                                  nc.scalar.copy(ob[:], ops3[:])
                              else:
                                  nc.vector.tensor_copy(ob[:], ops3[:])
                              nc.sync.dma_start(
                                  out_v[cb, :, dcol * 512:(dcol + 1) * 512], ob[:]
                              )
                  _PHASE_MARKS[f'end_b{b}'] = int(nc.get_next_instruction_name()[2:])
              _PHASE_MARKS['end'] = int(nc.get_next_instruction_name()[2:])
    nc.compile()
    return nc


# Permutation of the Dh dim: rotation-pair p = (2p, 2p+1) goes to partitions
# (qd*32 + j, qd*32 + 16 + j) with qd = p // 16, j = p % 16, so the
# real<->imag partner swap is a rotate-by-16 within each 32-partition quadrant
# (expressible as a DVE stream_shuffle).
_PERM = np.empty(Dh, dtype=np.int64)
_PAIR = np.empty(Dh, dtype=np.int64)   # rotation-pair index feeding each partition
_SGN = np.empty(Dh, dtype=np.float64)  # sign of the ri factor at each partition
for _qd in range(4):
    for _j in range(16):
        _p = _qd * 16 + _j
        _PERM[_qd * 32 + _j] = 2 * _p
        _PERM[_qd * 32 + 16 + _j] = 2 * _p + 1
        _PAIR[_qd * 32 + _j] = _p
        _PAIR[_qd * 32 + 16 + _j] = _p
        _SGN[_qd * 32 + _j] = -1.0
        _SGN[_qd * 32 + 16 + _j] = 1.0


def _prep_inputs(x, q_rope, k_rope, Wq, Wk, Wv, Wo):
    xt = np.ascontiguousarray(
        x.reshape(BN, D).T.astype(np.float16)
    )

    # rope factor tensors: per batch [qrA, qrB, krA, krB], each [128, N]
    s = float(Dh) ** -1.0  # both Dh^-0.5 scales folded into the q rope factors
    ropes = []
    for b in range(B):
        for r, scale in ((q_rope[b], s), (k_rope[b], 1.0)):
            rr = r[:, 0::2].T * scale   # [64, N], indexed by rotation pair
            ri = r[:, 1::2].T * scale
            ropes.append(rr[_PAIR])                  # A: rr at both partners
            ropes.append(ri[_PAIR] * _SGN[:, None])  # B: -ri at real, +ri at imag
    rope_all = np.ascontiguousarray(np.stack(ropes).astype(np.float16))

    in_maps = []
    for c in range(N_CORES):
        heads = range(HL * c, HL * (c + 1))
        wq_c = np.concatenate(
            [Wq[:, h * Dh:(h + 1) * Dh][:, _PERM] for h in heads], axis=1
        ).astype(np.float16)
        wk_c = np.concatenate(
            [Wk[:, h * Dh:(h + 1) * Dh][:, _PERM] for h in heads], axis=1
        ).astype(np.float16)
        wv_c = np.concatenate(
            [Wv[:, h * Dh:(h + 1) * Dh] for h in heads], axis=1
        ).astype(np.float16)
        wo_c = np.concatenate(
            [Wo[h * Dh:(h + 1) * Dh, :] for h in heads], axis=0
        ).astype(np.float16)
        in_maps.append(
            {
                "xt": xt,
                "wq": np.ascontiguousarray(wq_c),
                "wk": np.ascontiguousarray(wk_c),
                "wv": np.ascontiguousarray(wv_c),
                "wo": np.ascontiguousarray(wo_c),
                "rope": rope_all,
            }
        )
    return in_maps


def kernel(x, q_rope, k_rope, Wq, Wk, Wv, Wo, bo, **run_kwargs):
    if "nc" not in _CACHE:
        _CACHE["nc"] = _build_nc()
    nc = _CACHE["nc"]

    in_maps = _prep_inputs(x, q_rope, k_rope, Wq, Wk, Wv, Wo)
    res = run_bass_kernel_spmd(nc, in_maps, core_ids=list(range(N_CORES)), **run_kwargs)

    total = np.zeros((BN, D), dtype=np.float32)
    for c in range(N_CORES):
        total += res.results[c]["out"].astype(np.float32)
    total += bo.astype(np.float32)[None, :]
    out = total.reshape(B, N, D)
    _CACHE["last_res"] = res
    return out



# revision 6
# speedup vs baseline: 1.0695x; 1.0695x over previous
"""Trainium2 Bass kernel for AttentionWithRoPE, head-sharded across 8 NeuronCores.

Reference computation (fp32):
    q = (x @ Wq) * Dh^-0.5, rope'd; k = (x @ Wk), rope'd; v = x @ Wv
    out = softmax(q k^T * Dh^-0.5) v ;  final = out @ Wo + bo

Sharding: tensor-parallel over heads. Each core owns 2 of 16 heads: it gets
the full x (pre-transposed to [D, B*N] on host), its column slices of
Wq/Wk/Wv, its row slice of Wo, and returns a partial [B*N, D] output that the
host sums over cores (+ bo).

Device layout choices:
  - Q^T/K^T are produced directly in [Dh, n] layout (D-contraction with x^T as
    the moving operand), so attention scores can be computed transposed
    (S^T[m, n], K stationary / Q moving) and the exp'd probabilities feed the
    attn@V matmul as the moving operand with V (natural [m, Dh] layout) as
    stationary -- no on-chip transposes anywhere.
  - RoPE pairs (even, odd) are separated by permuting the columns of Wq/Wk
    per head on the host so that the (real, imag) partners of each rotation
    pair sit exactly 16 partitions apart within the same 32-partition
    quadrant. The partner swap is then a legal DVE stream_shuffle (rotation
    by 16 inside each quadrant); RoPE becomes shuffle + 3 vector ops per
    tile with host-prepared factor tensors A (rr replicated) and B (+-ri).
    Scores are permutation-invariant since q and k share the permutation;
    the total 1/Dh score scale is folded into the q-rope factors.
  - Softmax denominators: P^T column sums = (chain of DVE adds over the 16
    m-chunk tiles) then a ones-vector matmul on the PE for the partition-dim
    reduction; reciprocal on DVE; broadcast back over partitions with a K=1
    ones-row matmul on the PE.
  - All matmul operands fp16 (PE runs fp16 at full rate; fp32 would be 4x
    slower), accumulation in fp32 PSUM. Partial outputs returned fp16.
"""

import os
import sys

for _p in ("/opt/trn_rl_repo", "/root/.axon_site/_ro/trn_rl_repo"):
    if os.path.isdir(_p) and _p not in sys.path:
        sys.path.insert(0, _p)

import numpy as np
from contextlib import ExitStack

import concourse.bass as bass
import concourse.bacc as bacc
import concourse.tile as tile
from concourse import mybir
from concourse.bass_utils import run_bass_kernel_spmd

F16 = mybir.dt.float16
F32 = mybir.dt.float32
F8 = mybir.dt.float8e4
AF = mybir.ActivationFunctionType
DR = mybir.MatmulPerfMode.DoubleRow
WS = 32.0  # fp8 weight scale (W*0.02 would sit in e4m3 subnormal range)

N_CORES = 8
B, N, D, H, Dh = 2, 2048, 2048, 16, 128
HL = H // N_CORES          # heads per core
DHL = HL * Dh              # 256 local head dims
BN = B * N                 # 4096
DCH = D // 128             # 16 contraction chunks
NBLK = BN // 512           # 8 projection column blocks
MCH = N // 128             # 16 key chunks per sequence
NCK = N // 512             # 4 query chunks per sequence

_CACHE = {}
_PHASE_MARKS = {}


def _build_nc(loop_n=1):
    nc = bacc.Bacc(trn_type="TRN2", target_bir_lowering=False, debug=False)

    x8_d = nc.dram_tensor("x8", [D, BN], F8, kind="ExternalInput")
    xl8_d = nc.dram_tensor("xl8", [D, BN], F8, kind="ExternalInput")
    wq_d = nc.dram_tensor("wq", [D, DHL], F8, kind="ExternalInput")
    wk_d = nc.dram_tensor("wk", [D, DHL], F8, kind="ExternalInput")
    wvh_d = nc.dram_tensor("wvh", [D, DHL], F8, kind="ExternalInput")
    wvl_d = nc.dram_tensor("wvl", [D, DHL], F8, kind="ExternalInput")
    wo_d = nc.dram_tensor("wo", [DHL, D], F16, kind="ExternalInput")
    rope_d = nc.dram_tensor("rope", [2 * B * 2, 128, N], F16, kind="ExternalInput")
    out_d = nc.dram_tensor("out", [BN, D], F16, kind="ExternalOutput")

    x8_v = x8_d.ap().rearrange("(c p) n -> p c n", p=128)       # [128, 16, 4096]
    xl8_v = xl8_d.ap().rearrange("(c p) n -> p c n", p=128)
    w_views = {
        "wq": wq_d.ap().rearrange("(c p) m -> p c m", p=128),   # [128, 16, 256]
        "wk": wk_d.ap().rearrange("(c p) m -> p c m", p=128),
        "wvh": wvh_d.ap().rearrange("(c p) m -> p c m", p=128),
        "wvl": wvl_d.ap().rearrange("(c p) m -> p c m", p=128),
    }
    wo_v = wo_d.ap().rearrange("(j p) d -> p j d", p=128)       # [128, 2, 2048]
    rope_v = rope_d.ap()                                        # [8, 128, 2048]
    out_v = out_d.ap().rearrange("(cb p) d -> cb p d", p=128)   # [32, 128, 2048]

    with tile.TileContext(nc) as tc:
        with ExitStack() as ctx:
            consts = ctx.enter_context(tc.tile_pool(name="consts", bufs=1))
            qtkt = ctx.enter_context(tc.tile_pool(name="qtkt", bufs=1))
            vres = ctx.enter_context(tc.tile_pool(name="vres", bufs=1))
            xin = ctx.enter_context(tc.tile_pool(name="xin", bufs=2))
            ropein = ctx.enter_context(tc.tile_pool(name="ropein", bufs=2))
            tmps = ctx.enter_context(tc.tile_pool(name="tmps", bufs=3))
            ptile = ctx.enter_context(tc.tile_pool(name="ptile", bufs=12))
            dacc = ctx.enter_context(tc.tile_pool(name="dacc", bufs=2))
            smalls = ctx.enter_context(tc.tile_pool(name="smalls", bufs=2))
            rbcp = ctx.enter_context(tc.tile_pool(name="rbcp", bufs=2))
            otbuf = ctx.enter_context(tc.tile_pool(name="otbuf", bufs=3))
            obuf = ctx.enter_context(tc.tile_pool(name="obuf", bufs=4))

            psa = ctx.enter_context(tc.tile_pool(name="psa", bufs=2, space="PSUM"))
            psb = ctx.enter_context(tc.tile_pool(name="psb", bufs=3, space="PSUM"))
            psc = ctx.enter_context(tc.tile_pool(name="psc", bufs=1, space="PSUM"))

            # ---- resident weights / constants ----
            w_sb = {}
            for wname in ("wq", "wk", "wvh", "wvl"):
                w_sb[wname] = consts.tile([128, DCH, DHL], F8, name=wname)

            def _load_w(wname):
                for dq in range(4):
                    nc.sync.dma_start(
                        w_sb[wname][:, dq * 4:(dq + 1) * 4, :],
                        w_views[wname][:, dq * 4:(dq + 1) * 4, :],
                    )
            _load_w("wq")
            wo_sb = consts.tile([128, HL, D], F16, name="wo")
            if loop_n > 1:
                nc.sync.dma_start(wo_sb[:], wo_v)
            ones_col = consts.tile([128, 1], F16, name="ones_col")
            nc.vector.memset(ones_col[:], 1.0)
            ones_row = consts.tile([1, 128], F16, name="ones_row")
            nc.vector.memset(ones_row[:], 1.0)
            swap_mask = [(i + 16) % 32 for i in range(32)]

            qt_sb = qtkt.tile([128, HL, BN], F16, name="qt")
            kt_sb = qtkt.tile([128, HL, BN], F16, name="kt")
            v_sb = vres.tile([128, BN // 128, DHL], F16, name="v")

            # ---- phase 1: projections + rope ----
            import contextlib
            loop_cm = tc.For_i(0, loop_n, 1) if loop_n > 1 else contextlib.nullcontext()
            with loop_cm:
              for blk in range(NBLK):
                  b = blk // (NBLK // B)
                  c0 = (blk % (NBLK // B)) * 512
                  xblk = xin.tile([128, DCH, 512], F8, name="xblk")
                  xlblk = xin.tile([128, DCH, 512], F8, name="xlblk")
                  for dq in range(4):
                      nc.sync.dma_start(
                          xblk[:, dq * 4:(dq + 1) * 4, :],
                          x8_v[:, dq * 4:(dq + 1) * 4, blk * 512:(blk + 1) * 512],
                      )
                      nc.sync.dma_start(
                          xlblk[:, dq * 4:(dq + 1) * 4, :],
                          xl8_v[:, dq * 4:(dq + 1) * 4, blk * 512:(blk + 1) * 512],
                      )
                  rblk = ropein.tile([128, 4, 512], F16, name="rblk")
                  nc.sync.dma_start(
                      rblk[:], rope_v[4 * b:4 * b + 4, :, c0:c0 + 512].rearrange("r p n -> p r n")
                  )
                  if blk == 0:
                      _load_w("wk")
                      _load_w("wvh")
                      _load_w("wvl")

                  for wname, dst_sb, ra, rb_ in (
                      ("wq", qt_sb, 0, 1),
                      ("wk", kt_sb, 2, 3),
                  ):
                      for j in range(HL):
                          ps = psa.tile([128, 512], F32, name="pp")
                          for dc in range(DCH // 2):
                              nc.tensor.matmul(
                                  ps[:],
                                  w_sb[wname][:, 2 * dc:2 * dc + 2, j * 128:(j + 1) * 128],
                                  xblk[:, 2 * dc:2 * dc + 2, :],
                                  start=(dc == 0),
                                  stop=(dc == DCH // 2 - 1),
                                  perf_mode=DR,
                              )
                          raw = tmps.tile([128, 512], F16, name="raw")
                          nc.scalar.copy(raw[:], ps[:])
                          t2 = tmps.tile([128, 512], F16, name="t2")
                          nc.vector.stream_shuffle(t2[:], raw[:], swap_mask)
                          nc.vector.tensor_mul(t2[:], t2[:], rblk[:, rb_, :])
                          nc.vector.tensor_mul(raw[:], raw[:], rblk[:, ra, :])
                          nc.vector.tensor_add(
                              dst_sb[:, j, blk * 512:(blk + 1) * 512], raw[:], t2[:]
                          )

                  for mc in range(4):
                      psv = psb.tile([128, DHL], F32, name="pb")
                      nmm = 3 * (DCH // 2)
                      i = 0
                      for xsrc, wname in ((xblk, "wvh"), (xblk, "wvl"), (xlblk, "wvh")):
                          for dc in range(DCH // 2):
                              nc.tensor.matmul(
                                  psv[:],
                                  xsrc[:, 2 * dc:2 * dc + 2, mc * 128:(mc + 1) * 128],
                                  w_sb[wname][:, 2 * dc:2 * dc + 2, :],
                                  start=(i == 0),
                                  stop=(i == nmm - 1),
                                  perf_mode=DR,
                              )
                              i += 1
                      nc.scalar.activation(
                          v_sb[:, blk * 4 + mc, :], psv[:], AF.Copy, scale=1.0 / WS
                      )

              if loop_n == 1:
                  nc.sync.dma_start(wo_sb[:], wo_v)
              _PHASE_MARKS['end_phase1'] = int(nc.get_next_instruction_name()[2:])
              # ---- phase 2+3 per batch ----
              for b in range(B):
                  ot_tiles = [otbuf.tile([128, N], F16, name="ot") for _ in range(HL)]
                  for nck in range(NCK):
                      nq0 = b * N + nck * 512
                      for j in range(HL):
                          ot = ot_tiles[j]
                          pts = []
                          for mc2 in range(MCH // 2):
                              sp = psa.tile([128, 1024], F32, name="pp")
                              for half in range(2):
                                  mc = 2 * mc2 + half
                                  nc.tensor.matmul(
                                      sp[:, half * 512:(half + 1) * 512],
                                      kt_sb[:, j, b * N + mc * 128:b * N + (mc + 1) * 128],
                                      qt_sb[:, j, nq0:nq0 + 512],
                                      start=True,
                                      stop=True,
                                  )
                              pt = ptile.tile([128, 1024], F16, name="pt")
                              nc.scalar.activation(pt[:], sp[:], AF.Exp)
                              pts.append(pt)

                          op = psb.tile([128, 512], F32, name="pb")
                          for mc in range(MCH):
                              nc.tensor.matmul(
                                  op[:],
                                  v_sb[:, b * MCH + mc, j * 128:(j + 1) * 128],
                                  pts[mc // 2][:, (mc % 2) * 512:(mc % 2 + 1) * 512],
                                  start=(mc == 0),
                                  stop=(mc == MCH - 1),
                              )

                          acc = dacc.tile([128, 1024], F16, name="acc")
                          nc.vector.tensor_add(acc[:], pts[0][:], pts[1][:])
                          for mc2 in range(2, MCH // 2):
                              nc.vector.tensor_add(acc[:], acc[:], pts[mc2][:])
                          nc.vector.tensor_add(
                              acc[:, 0:512], acc[:, 0:512], acc[:, 512:1024]
                          )
                          dps = psc.tile([1, 512], F32, name="pc")
                          nc.tensor.matmul(
                              dps[:], ones_col[:], acc[:, 0:512], start=True, stop=True
                          )
                          rc32 = smalls.tile([1, 512], F32, name="rc32")
                          nc.vector.reciprocal(rc32[:], dps[:])
                          rc16 = smalls.tile([1, 512], F16, name="rc16")
                          nc.vector.tensor_copy(rc16[:], rc32[:])
                          bps = psc.tile([128, 512], F32, name="pc")
                          nc.tensor.matmul(bps[:], ones_row[:], rc16[:], start=True, stop=True)
                          rbc = rbcp.tile([128, 512], F16, name="rbc")
                          nc.vector.tensor_copy(rbc[:], bps[:])
                          nc.vector.tensor_mul(ot[:, nck * 512:(nck + 1) * 512], op[:], rbc[:])

                      # output projection for the n-range this nck covers
                      for nck2 in range(4):
                          ncol = nck * 4 + nck2
                          cb = b * (N // 128) + ncol
                          for dcol in range(D // 512):
                              ops3 = psb.tile([128, 512], F32, name="pb")
                              for j in range(HL):
                                  nc.tensor.matmul(
                                      ops3[:],
                                      ot_tiles[j][:, ncol * 128:(ncol + 1) * 128],
                                      wo_sb[:, j, dcol * 512:(dcol + 1) * 512],
                                      start=(j == 0),
                                      stop=(j == HL - 1),
                                  )
                              ob = obuf.tile([128, 512], F16, name="ob")
                              if dcol % 2 == 0:
                                  nc.scalar.copy(ob[:], ops3[:])
                              else:
                                  nc.vector.tensor_copy(ob[:], ops3[:])
                              nc.sync.dma_start(
                                  out_v[cb, :, dcol * 512:(dcol + 1) * 512], ob[:]
                              )
                  _PHASE_MARKS[f'end_b{b}'] = int(nc.get_next_instruction_name()[2:])
              _PHASE_MARKS['end'] = int(nc.get_next_instruction_name()[2:])
    nc.compile()
    return nc


# Permutation of the Dh dim: rotation-pair p = (2p, 2p+1) goes to partitions
# (qd*32 + j, qd*32 + 16 + j) with qd = p // 16, j = p % 16, so the
# real<->imag partner swap is a rotate-by-16 within each 32-partition quadrant
# (expressible as a DVE stream_shuffle).
_PERM = np.empty(Dh, dtype=np.int64)
_PAIR = np.empty(Dh, dtype=np.int64)   # rotation-pair index feeding each partition
_SGN = np.empty(Dh, dtype=np.float64)  # sign of the ri factor at each partition
for _qd in range(4):
    for _j in range(16):
        _p = _qd * 16 + _j
        _PERM[_qd * 32 + _j] = 2 * _p
        _PERM[_qd * 32 + 16 + _j] = 2 * _p + 1
        _PAIR[_qd * 32 + _j] = _p
        _PAIR[_qd * 32 + 16 + _j] = _p
        _SGN[_qd * 32 + _j] = -1.0
        _SGN[_qd * 32 + 16 + _j] = 1.0


def _prep_inputs(x, q_rope, k_rope, Wq, Wk, Wv, Wo):
    import ml_dtypes

    NF8 = ml_dtypes.float8_e4m3
    xt = x.reshape(BN, D).T.astype(np.float32)
    x8 = xt.astype(NF8)
    xl8 = (xt - x8.astype(np.float32)).astype(NF8)
    x8 = np.ascontiguousarray(x8)
    xl8 = np.ascontiguousarray(xl8)

    # rope factor tensors: per batch [qrA, qrB, krA, krB], each [128, N].
    # Both Dh^-0.5 score scales fold into the q factors; both q and k factors
    # additionally carry 1/WS to undo the fp8 weight scaling.
    s = float(Dh) ** -1.0
    ropes = []
    for b in range(B):
        for r, scale in ((q_rope[b], s / WS), (k_rope[b], 1.0 / WS)):
            rr = r[:, 0::2].T * scale   # [64, N], indexed by rotation pair
            ri = r[:, 1::2].T * scale
            ropes.append(rr[_PAIR])                  # A: rr at both partners
            ropes.append(ri[_PAIR] * _SGN[:, None])  # B: -ri at real, +ri at imag
    rope_all = np.ascontiguousarray(np.stack(ropes).astype(np.float16))

    in_maps = []
    for c in range(N_CORES):
        heads = range(HL * c, HL * (c + 1))
        wq_c = np.concatenate(
            [Wq[:, h * Dh:(h + 1) * Dh][:, _PERM] for h in heads], axis=1
        ).astype(np.float32) * WS
        wk_c = np.concatenate(
            [Wk[:, h * Dh:(h + 1) * Dh][:, _PERM] for h in heads], axis=1
        ).astype(np.float32) * WS
        wv_c = np.concatenate(
            [Wv[:, h * Dh:(h + 1) * Dh] for h in heads], axis=1
        ).astype(np.float32) * WS
        wvh_c = wv_c.astype(NF8)
        wvl_c = (wv_c - wvh_c.astype(np.float32)).astype(NF8)
        wo_c = np.concatenate(
            [Wo[h * Dh:(h + 1) * Dh, :] for h in heads], axis=0
        ).astype(np.float16)
        in_maps.append(
            {
                "x8": x8,
                "xl8": xl8,
                "wq": np.ascontiguousarray(wq_c.astype(NF8)),
                "wk": np.ascontiguousarray(wk_c.astype(NF8)),
                "wvh": np.ascontiguousarray(wvh_c),
                "wvl": np.ascontiguousarray(wvl_c),
                "wo": np.ascontiguousarray(wo_c),
                "rope": rope_all,
            }
        )
    return in_maps


def kernel(x, q_rope, k_rope, Wq, Wk, Wv, Wo, bo, **run_kwargs):
    if "nc" not in _CACHE:
        _CACHE["nc"] = _build_nc()
    nc = _CACHE["nc"]

    in_maps = _prep_inputs(x, q_rope, k_rope, Wq, Wk, Wv, Wo)
    res = run_bass_kernel_spmd(nc, in_maps, core_ids=list(range(N_CORES)), **run_kwargs)

    total = np.zeros((BN, D), dtype=np.float32)
    for c in range(N_CORES):
        total += res.results[c]["out"].astype(np.float32)
    total += bo.astype(np.float32)[None, :]
    out = total.reshape(B, N, D)
    _CACHE["last_res"] = res
    return out



# revision 8
# speedup vs baseline: 6.0866x; 5.6911x over previous
"""Trainium2 Bass kernel for AttentionWithRoPE, head-sharded across 8 NeuronCores.

Reference computation (fp32):
    q = (x @ Wq) * Dh^-0.5, rope'd; k = (x @ Wk), rope'd; v = x @ Wv
    out = softmax(q k^T * Dh^-0.5) v ;  final = out @ Wo + bo

Sharding: tensor-parallel over heads. Each core owns 2 of 16 heads: it gets
the full x (pre-transposed to [D, B*N] on host), its column slices of
Wq/Wk/Wv, its row slice of Wo, and returns a partial [B*N, D] output that the
host sums over cores (+ bo).

Device layout choices:
  - Q^T/K^T are produced directly in [Dh, n] layout (D-contraction with x^T as
    the moving operand), so attention scores can be computed transposed
    (S^T[m, n], K stationary / Q moving) and the exp'd probabilities feed the
    attn@V matmul as the moving operand with V (natural [m, Dh] layout) as
    stationary -- no on-chip transposes anywhere.
  - RoPE pairs (even, odd) are separated by permuting the columns of Wq/Wk
    per head on the host so that the (real, imag) partners of each rotation
    pair sit exactly 16 partitions apart within the same 32-partition
    quadrant. The partner swap is then a legal DVE stream_shuffle (rotation
    by 16 inside each quadrant); RoPE becomes shuffle + 3 vector ops per
    tile with host-prepared factor tensors A (rr replicated) and B (+-ri).
    Scores are permutation-invariant since q and k share the permutation;
    the total 1/Dh score scale is folded into the q-rope factors.
  - Softmax denominators: P^T column sums = (chain of DVE adds over the 16
    m-chunk tiles) then a ones-vector matmul on the PE for the partition-dim
    reduction; reciprocal on DVE; broadcast back over partitions with a K=1
    ones-row matmul on the PE.
  - All matmul operands fp16 (PE runs fp16 at full rate; fp32 would be 4x
    slower), accumulation in fp32 PSUM. Partial outputs returned fp16.
"""

import os
import sys

for _p in ("/opt/trn_rl_repo", "/root/.axon_site/_ro/trn_rl_repo"):
    if os.path.isdir(_p) and _p not in sys.path:
        sys.path.insert(0, _p)

import numpy as np
from contextlib import ExitStack

import concourse.bass as bass
import concourse.bacc as bacc
import concourse.tile as tile
from concourse import mybir
from concourse.bass_utils import run_bass_kernel_spmd

F16 = mybir.dt.float16
F32 = mybir.dt.float32
F8 = mybir.dt.float8e4
AF = mybir.ActivationFunctionType
DR = mybir.MatmulPerfMode.DoubleRow
WS = 32.0  # fp8 weight scale (W*0.02 would sit in e4m3 subnormal range)

N_CORES = 8
B, N, D, H, Dh = 2, 2048, 2048, 16, 128
HL = H // N_CORES          # heads per core
DHL = HL * Dh              # 256 local head dims
BN = B * N                 # 4096
DCH = D // 128             # 16 contraction chunks
NBLK = BN // 512           # 8 projection column blocks
MCH = N // 128             # 16 key chunks per sequence
NCK = N // 512             # 4 query chunks per sequence

_CACHE = {}
_PHASE_MARKS = {}


def _build_nc(loop_n=1, p1_only=False):
    nc = bacc.Bacc(trn_type="TRN2", target_bir_lowering=False, debug=False)

    x8_d = nc.dram_tensor("x8", [D, BN], F8, kind="ExternalInput")
    xl8_d = nc.dram_tensor("xl8", [D, BN], F8, kind="ExternalInput")
    wq_d = nc.dram_tensor("wq", [D, DHL], F8, kind="ExternalInput")
    wk_d = nc.dram_tensor("wk", [D, DHL], F8, kind="ExternalInput")
    wvh_d = nc.dram_tensor("wvh", [D, DHL], F8, kind="ExternalInput")
    wvl_d = nc.dram_tensor("wvl", [D, DHL], F8, kind="ExternalInput")
    wo_d = nc.dram_tensor("wo", [DHL, D], F16, kind="ExternalInput")
    rope_d = nc.dram_tensor("rope", [2 * B * 2, 128, N], F16, kind="ExternalInput")
    out_d = nc.dram_tensor("out", [BN, D], F16, kind="ExternalOutput")

    x8_v = x8_d.ap().rearrange("(c p) n -> p c n", p=128)       # [128, 16, 4096]
    xl8_v = xl8_d.ap().rearrange("(c p) n -> p c n", p=128)
    w_views = {
        "wq": wq_d.ap().rearrange("(c p) m -> p c m", p=128),   # [128, 16, 256]
        "wk": wk_d.ap().rearrange("(c p) m -> p c m", p=128),
        "wvh": wvh_d.ap().rearrange("(c p) m -> p c m", p=128),
        "wvl": wvl_d.ap().rearrange("(c p) m -> p c m", p=128),
    }
    wo_v = wo_d.ap().rearrange("(j p) d -> p j d", p=128)       # [128, 2, 2048]
    rope_v = rope_d.ap()                                        # [8, 128, 2048]
    out_v = out_d.ap().rearrange("(cb p) d -> cb p d", p=128)   # [32, 128, 2048]

    with tile.TileContext(nc) as tc:
        with ExitStack() as ctx:
            consts = ctx.enter_context(tc.tile_pool(name="consts", bufs=1))
            qtkt = ctx.enter_context(tc.tile_pool(name="qtkt", bufs=1))
            vres = ctx.enter_context(tc.tile_pool(name="vres", bufs=1))
            xin = ctx.enter_context(tc.tile_pool(name="xin", bufs=2))
            ropein = ctx.enter_context(tc.tile_pool(name="ropein", bufs=2))
            tmps = ctx.enter_context(tc.tile_pool(name="tmps", bufs=3))
            ptile = ctx.enter_context(tc.tile_pool(name="ptile", bufs=12))
            dacc = ctx.enter_context(tc.tile_pool(name="dacc", bufs=2))
            smalls = ctx.enter_context(tc.tile_pool(name="smalls", bufs=2))
            rbcp = ctx.enter_context(tc.tile_pool(name="rbcp", bufs=2))
            otbuf = ctx.enter_context(tc.tile_pool(name="otbuf", bufs=3))
            obuf = ctx.enter_context(tc.tile_pool(name="obuf", bufs=4))

            psa = ctx.enter_context(tc.tile_pool(name="psa", bufs=2, space="PSUM"))
            psb = ctx.enter_context(tc.tile_pool(name="psb", bufs=3, space="PSUM"))
            psc = ctx.enter_context(tc.tile_pool(name="psc", bufs=1, space="PSUM"))

            # ---- resident weights / constants ----
            w_sb = {}
            for wname in ("wq", "wk", "wvh", "wvl"):
                w_sb[wname] = consts.tile([128, DCH, DHL], F8, name=wname)

            def _load_w(wname):
                for dq in range(4):
                    nc.sync.dma_start(
                        w_sb[wname][:, dq * 4:(dq + 1) * 4, :],
                        w_views[wname][:, dq * 4:(dq + 1) * 4, :],
                    )
            _load_w("wq")
            wo_sb = consts.tile([128, HL, D], F16, name="wo")
            if loop_n > 1:
                nc.sync.dma_start(wo_sb[:], wo_v)
            ones_col = consts.tile([128, 1], F16, name="ones_col")
            nc.vector.memset(ones_col[:], 1.0)
            ones_row = consts.tile([1, 128], F16, name="ones_row")
            nc.vector.memset(ones_row[:], 1.0)
            swap_mask = [(i + 16) % 32 for i in range(32)]

            qt_sb = qtkt.tile([128, HL, BN], F16, name="qt")
            kt_sb = qtkt.tile([128, HL, BN], F16, name="kt")
            v_sb = vres.tile([128, BN // 128, DHL], F16, name="v")

            # ---- phase 1: projections + rope ----
            import contextlib
            loop_cm = tc.For_i(0, loop_n, 1) if loop_n > 1 else contextlib.nullcontext()
            with loop_cm:
              for blk in range(NBLK):
                  b = blk // (NBLK // B)
                  c0 = (blk % (NBLK // B)) * 512
                  xblk = xin.tile([128, DCH, 512], F8, name="xblk")
                  xlblk = xin.tile([128, DCH, 512], F8, name="xlblk")
                  for dq in range(4):
                      nc.sync.dma_start(
                          xblk[:, dq * 4:(dq + 1) * 4, :],
                          x8_v[:, dq * 4:(dq + 1) * 4, blk * 512:(blk + 1) * 512],
                      )
                      nc.sync.dma_start(
                          xlblk[:, dq * 4:(dq + 1) * 4, :],
                          xl8_v[:, dq * 4:(dq + 1) * 4, blk * 512:(blk + 1) * 512],
                      )
                  rblk = ropein.tile([128, 4, 512], F16, name="rblk")
                  nc.sync.dma_start(
                      rblk[:], rope_v[4 * b:4 * b + 4, :, c0:c0 + 512].rearrange("r p n -> p r n")
                  )
                  if blk == 0:
                      _load_w("wk")
                      _load_w("wvh")
                      _load_w("wvl")

                  for wname, dst_sb, ra, rb_ in (
                      ("wq", qt_sb, 0, 1),
                      ("wk", kt_sb, 2, 3),
                  ):
                      for j in range(HL):
                          ps = psa.tile([128, 512], F32, name="pp")
                          for dc in range(DCH // 2):
                              nc.tensor.matmul(
                                  ps[:],
                                  w_sb[wname][:, 2 * dc:2 * dc + 2, j * 128:(j + 1) * 128],
                                  xblk[:, 2 * dc:2 * dc + 2, :],
                                  start=(dc == 0),
                                  stop=(dc == DCH // 2 - 1),
                                  perf_mode=DR,
                              )
                          raw = tmps.tile([128, 512], F16, name="raw")
                          nc.scalar.copy(raw[:], ps[:])
                          t2 = tmps.tile([128, 512], F16, name="t2")
                          nc.vector.stream_shuffle(t2[:], raw[:], swap_mask)
                          nc.vector.tensor_mul(t2[:], t2[:], rblk[:, rb_, :])
                          nc.vector.tensor_mul(raw[:], raw[:], rblk[:, ra, :])
                          nc.vector.tensor_add(
                              dst_sb[:, j, blk * 512:(blk + 1) * 512], raw[:], t2[:]
                          )

                  for mc in range(4):
                      psv = psb.tile([128, DHL], F32, name="pb")
                      nmm = 3 * (DCH // 2)
                      i = 0
                      for xsrc, wname in ((xblk, "wvh"), (xblk, "wvl"), (xlblk, "wvh")):
                          for dc in range(DCH // 2):
                              nc.tensor.matmul(
                                  psv[:],
                                  xsrc[:, 2 * dc:2 * dc + 2, mc * 128:(mc + 1) * 128],
                                  w_sb[wname][:, 2 * dc:2 * dc + 2, :],
                                  start=(i == 0),
                                  stop=(i == nmm - 1),
                                  perf_mode=DR,
                              )
                              i += 1
                      nc.scalar.activation(
                          v_sb[:, blk * 4 + mc, :], psv[:], AF.Copy, scale=1.0 / WS
                      )

              if loop_n == 1:
                  nc.sync.dma_start(wo_sb[:], wo_v)
              _PHASE_MARKS['end_phase1'] = int(nc.get_next_instruction_name()[2:])
              # ---- phase 2+3 per batch ----
              for b in range(B if not p1_only else 0):
                  ot_tiles = [otbuf.tile([128, N], F16, name="ot") for _ in range(HL)]
                  for nck in range(NCK):
                      nq0 = b * N + nck * 512
                      for j in range(HL):
                          ot = ot_tiles[j]
                          pts = []
                          for mc2 in range(MCH // 2):
                              sp = psa.tile([128, 1024], F32, name="pp")
                              for half in range(2):
                                  mc = 2 * mc2 + half
                                  nc.tensor.matmul(
                                      sp[:, half * 512:(half + 1) * 512],
                                      kt_sb[:, j, b * N + mc * 128:b * N + (mc + 1) * 128],
                                      qt_sb[:, j, nq0:nq0 + 512],
                                      start=True,
                                      stop=True,
                                  )
                              pt = ptile.tile([128, 1024], F16, name="pt")
                              nc.scalar.activation(pt[:], sp[:], AF.Exp)
                              pts.append(pt)

                          op = psb.tile([128, 512], F32, name="pb")
                          for mc in range(MCH):
                              nc.tensor.matmul(
                                  op[:],
                                  v_sb[:, b * MCH + mc, j * 128:(j + 1) * 128],
                                  pts[mc // 2][:, (mc % 2) * 512:(mc % 2 + 1) * 512],
                                  start=(mc == 0),
                                  stop=(mc == MCH - 1),
                              )

                          acc = dacc.tile([128, 1024], F16, name="acc")
                          nc.vector.tensor_add(acc[:], pts[0][:], pts[1][:])
                          for mc2 in range(2, MCH // 2):
                              nc.vector.tensor_add(acc[:], acc[:], pts[mc2][:])
                          nc.vector.tensor_add(
                              acc[:, 0:512], acc[:, 0:512], acc[:, 512:1024]
                          )
                          dps = psc.tile([1, 512], F32, name="pc")
                          nc.tensor.matmul(
                              dps[:], ones_col[:], acc[:, 0:512], start=True, stop=True
                          )
                          rc32 = smalls.tile([1, 512], F32, name="rc32")
                          nc.vector.reciprocal(rc32[:], dps[:])
                          rc16 = smalls.tile([1, 512], F16, name="rc16")
                          nc.vector.tensor_copy(rc16[:], rc32[:])
                          bps = psc.tile([128, 512], F32, name="pc")
                          nc.tensor.matmul(bps[:], ones_row[:], rc16[:], start=True, stop=True)
                          rbc = rbcp.tile([128, 512], F16, name="rbc")
                          nc.vector.tensor_copy(rbc[:], bps[:])
                          nc.vector.tensor_mul(ot[:, nck * 512:(nck + 1) * 512], op[:], rbc[:])

                      # output projection for the n-range this nck covers
                      for nck2 in range(4):
                          ncol = nck * 4 + nck2
                          cb = b * (N // 128) + ncol
                          for dcol in range(D // 512):
                              ops3 = psb.tile([128, 512], F32, name="pb")
                              for j in range(HL):
                                  nc.tensor.matmul(
                                      ops3[:],
                                      ot_tiles[j][:, ncol * 128:(ncol + 1) * 128],
                                      wo_sb[:, j, dcol * 512:(dcol + 1) * 512],
                                      start=(j == 0),
                                      stop=(j == HL - 1),
                                  )
                              ob = obuf.tile([128, 512], F16, name="ob")
                              if dcol % 2 == 0:
                                  nc.scalar.copy(ob[:], ops3[:])
                              else:
                                  nc.vector.tensor_copy(ob[:], ops3[:])
                              nc.sync.dma_start(
                                  out_v[cb, :, dcol * 512:(dcol + 1) * 512], ob[:]
                              )
                  _PHASE_MARKS[f'end_b{b}'] = int(nc.get_next_instruction_name()[2:])
              _PHASE_MARKS['end'] = int(nc.get_next_instruction_name()[2:])
    nc.compile()
    return nc


# Permutation of the Dh dim: rotation-pair p = (2p, 2p+1) goes to partitions
# (qd*32 + j, qd*32 + 16 + j) with qd = p // 16, j = p % 16, so the
# real<->imag partner swap is a rotate-by-16 within each 32-partition quadrant
# (expressible as a DVE stream_shuffle).
_PERM = np.empty(Dh, dtype=np.int64)
_PAIR = np.empty(Dh, dtype=np.int64)   # rotation-pair index feeding each partition
_SGN = np.empty(Dh, dtype=np.float64)  # sign of the ri factor at each partition
for _qd in range(4):
    for _j in range(16):
        _p = _qd * 16 + _j
        _PERM[_qd * 32 + _j] = 2 * _p
        _PERM[_qd * 32 + 16 + _j] = 2 * _p + 1
        _PAIR[_qd * 32 + _j] = _p
        _PAIR[_qd * 32 + 16 + _j] = _p
        _SGN[_qd * 32 + _j] = -1.0
        _SGN[_qd * 32 + 16 + _j] = 1.0


def _prep_inputs(x, q_rope, k_rope, Wq, Wk, Wv, Wo):
    import ml_dtypes

    NF8 = ml_dtypes.float8_e4m3
    xt = x.reshape(BN, D).T.astype(np.float32)
    x8 = xt.astype(NF8)
    xl8 = (xt - x8.astype(np.float32)).astype(NF8)
    x8 = np.ascontiguousarray(x8)
    xl8 = np.ascontiguousarray(xl8)

    # rope factor tensors: per batch [qrA, qrB, krA, krB], each [128, N].
    # Both Dh^-0.5 score scales fold into the q factors; both q and k factors
    # additionally carry 1/WS to undo the fp8 weight scaling.
    s = float(Dh) ** -1.0
    ropes = []
    for b in range(B):
        for r, scale in ((q_rope[b], s / WS), (k_rope[b], 1.0 / WS)):
            rr = r[:, 0::2].T * scale   # [64, N], indexed by rotation pair
            ri = r[:, 1::2].T * scale
            ropes.append(rr[_PAIR])                  # A: rr at both partners
            ropes.append(ri[_PAIR] * _SGN[:, None])  # B: -ri at real, +ri at imag
    rope_all = np.ascontiguousarray(np.stack(ropes).astype(np.float16))

    in_maps = []
    for c in range(N_CORES):
        heads = range(HL * c, HL * (c + 1))
        wq_c = np.concatenate(
            [Wq[:, h * Dh:(h + 1) * Dh][:, _PERM] for h in heads], axis=1
        ).astype(np.float32) * WS
        wk_c = np.concatenate(
            [Wk[:, h * Dh:(h + 1) * Dh][:, _PERM] for h in heads], axis=1
        ).astype(np.float32) * WS
        wv_c = np.concatenate(
            [Wv[:, h * Dh:(h + 1) * Dh] for h in heads], axis=1
        ).astype(np.float32) * WS
        wvh_c = wv_c.astype(NF8)
        wvl_c = (wv_c - wvh_c.astype(np.float32)).astype(NF8)
        wo_c = np.concatenate(
            [Wo[h * Dh:(h + 1) * Dh, :] for h in heads], axis=0
        ).astype(np.float16)
        in_maps.append(
            {
                "x8": x8,
                "xl8": xl8,
                "wq": np.ascontiguousarray(wq_c.astype(NF8)),
                "wk": np.ascontiguousarray(wk_c.astype(NF8)),
                "wvh": np.ascontiguousarray(wvh_c),
                "wvl": np.ascontiguousarray(wvl_c),
                "wo": np.ascontiguousarray(wo_c),
                "rope": rope_all,
            }
        )
    return in_maps


def kernel(x, q_rope, k_rope, Wq, Wk, Wv, Wo, bo, **run_kwargs):
    if "nc" not in _CACHE:
        _CACHE["nc"] = _build_nc()
    nc = _CACHE["nc"]

    in_maps = _prep_inputs(x, q_rope, k_rope, Wq, Wk, Wv, Wo)
    res = run_bass_kernel_spmd(nc, in_maps, core_ids=list(range(N_CORES)), **run_kwargs)

    total = np.zeros((BN, D), dtype=np.float32)
    for c in range(N_CORES):
        total += res.results[c]["out"].astype(np.float32)
    total += bo.astype(np.float32)[None, :]
    out = total.reshape(B, N, D)
    _CACHE["last_res"] = res
    return out

